# revision 1
# baseline (speedup 1.0000x reference)
"""Bass/Tile kernel builder for the CRML loss function, SPMD over 8 cores.

Math (see reference): loss = loss0 + glove_u + alpha*mse_u + glove_i + beta*mse_i
Key facts used:
  - glove loss is invariant to a consistent permutation of the batch, so we
    skip the sort and use user_ids / pos_ids order everywhere.
  - co-occurrence GEMM inputs are 0/1 so bf16 matmul with fp32 PSUM is exact.
  - w (glove weight) is 0 wherever co<=1, so zero-intersection cells and the
    forced-diagonal cells drop out; the diagonal must be masked since raw
    diag counts are large. A per-core {0,1} mask input handles it.

Per-core sharding:
  - user side: K(=item axis)-split. Each core holds a 1/8 column window of
    the interactions matrix (padded to WIN cols). One dma_gather(transpose)
    pulls the 1024 user rows' window transposed into SBUF; 0/1 bytes are
    converted to bf16 and fed as both matmul operands. Partial (B,B) products
    are ReduceScattered so core c ends with rows [128c:128c+128) summed.
  - item side: K(=user axis)-split. Each core streams its row shard
    (RSH x NIP), gathers the 1024 pos_id columns per 128-row chunk with
    gpsimd.indirect_copy, converts to bf16, same matmul/RS pattern.
  - loss0 / mse / glove elementwise: batch-split, 128 rows per core.
Outputs: per-core partial sums vector; host combines (means + weights).
"""

from contextlib import ExitStack

import numpy as np

import concourse.bass as bass
import concourse.mybir as mybir
from concourse import tile

F32 = mybir.dt.float32
BF16 = mybir.dt.bfloat16
U8 = mybir.dt.uint8
I16 = mybir.dt.int16
U16 = mybir.dt.uint16
I32 = mybir.dt.int32

AF = mybir.ActivationFunctionType
ALU = mybir.AluOpType

MARGIN = 2.0
C_MAX = 100.0
LAM = 0.75

# partial-sum slots
SLOT_RELU = 0
SLOT_GLOVE_U = 1
SLOT_GLOVE_I = 2
SLOT_MSE_U = 3
SLOT_MSE_I = 4
N_SLOTS = 8


class Cfg:
    """Problem geometry. Full size: NU=NI=20000, B=1024, NEG=16, D=256,
    WIN=2560, RSH=2560, NIP=20480."""

    def __init__(self, NU=20000, NI=20000, B=1024, NEG=16, D=256,
                 WIN=2560, NIP=20480, n_cores=8, parts=15, no_cc=False,
                 reps=1):
        self.reps = reps  # body repetitions (timing only)
        # parts bitmask: 1=user gemm+RS, 2=item gemm+RS, 4=loss0/mse, 8=glove
        self.parts = parts
        self.no_cc = no_cc  # replace collectives with local copies (timing sims)
        self.NU, self.NI, self.B, self.NEG, self.D = NU, NI, B, NEG, D
        self.WIN = WIN          # user-side column window per core (mult of 256)
        self.NIP = NIP          # padded row length for item-side row shard
        self.RSH = NIP // n_cores  # rows per core in item-side shard (mult of 128)
        self.n_cores = n_cores
        assert WIN % 256 == 0 and WIN * n_cores >= NI
        assert NIP % 256 == 0 and NIP >= NI and self.RSH % 128 == 0
        assert B % (128 * 1) == 0
        self.MB = B // 128      # number of 128-row M blocks
        self.BPC = B // n_cores  # batch rows per core (=128 at full size)
        assert self.BPC == 128, "kernel assumes 128 batch rows per core"
        self.NBLK = B // 512 if B % 512 == 0 else None
        assert B % 512 == 0
        self.KU = WIN // 128    # user-side K chunks per core
        self.KI = self.RSH // 128  # item-side K chunks per core


def wrap16(ids: np.ndarray, dtype) -> np.ndarray:
    """Wrapped-by-16 index layout for dma_gather / indirect_copy:
    out[p, s] = ids[s*16 + p%16], replicated across the 8 groups."""
    n = len(ids)
    assert n % 16 == 0
    out = np.zeros((128, n // 16), dtype=dtype)
    for p in range(128):
        out[p, :] = ids[np.arange(n // 16) * 16 + p % 16]
    return out


def build_host_inputs(cfg: Cfg, inputs: dict) -> list[dict]:
    """Shard/layout the full problem inputs into per-core input maps."""
    c = cfg
    I8 = np.ascontiguousarray(inputs["interactions"]).view(np.uint8)
    assert I8.shape == (c.NU, c.NI)
    user_ids = np.asarray(inputs["user_ids"]).astype(np.int64)
    pos_ids = np.asarray(inputs["pos_ids"]).astype(np.int64)
    neg_ids = np.asarray(inputs["neg_ids"]).astype(np.int64)

    uemb = np.ascontiguousarray(inputs["user_emb"])
    iemb = np.ascontiguousarray(inputs["item_emb"])
    cuemb = np.ascontiguousarray(inputs["co_user_emb"])
    ciemb = np.ascontiguousarray(inputs["co_item_emb"])
    cub = np.ascontiguousarray(inputs["co_user_bias"])
    cib = np.ascontiguousarray(inputs["co_item_bias"])

    uball = user_ids.reshape(c.MB, 128).T.astype(np.int32)   # [128, MB]
    iball = pos_ids.reshape(c.MB, 128).T.astype(np.int32)

    in_maps = []
    for core in range(c.n_cores):
        # user-side column window shard, zero padded
        iw = np.zeros((c.NU, c.WIN), np.uint8)
        lo = core * c.WIN
        hi = min(c.NI, lo + c.WIN)
        if hi > lo:
            iw[:, : hi - lo] = I8[:, lo:hi]
        # item-side: window shard of I^T (host-transposed layout choice):
        # itwin[i, ul] = I[core*WIN + ul, i]
        itwin = np.zeros((c.NI, c.WIN), np.uint8)
        rlo = core * c.WIN
        rhi = min(c.NU, rlo + c.WIN)
        if rhi > rlo:
            itwin[:, : rhi - rlo] = I8[rlo:rhi, :].T
        # diag mask: 0 at [p, 128*core+p] (valid only when this core's rows
        # are real batch rows, which they always are: B/8 = 128)
        diag_off = np.ones((128, c.B), np.float32)
        diag_off[np.arange(128), 128 * core + np.arange(128)] = 0.0
        sl = slice(core * 128, core * 128 + 128)
        in_maps.append({
            "iw": iw,
            "itwin": itwin,
            "diag_off": diag_off,
            "ids_c": user_ids[sl].astype(np.int32).reshape(128, 1),
            "pids_c": pos_ids[sl].astype(np.int32).reshape(128, 1),
            "nids_c": neg_ids[sl].astype(np.int32),
            "uball": uball,
            "iball": iball,
            "user_emb": uemb,
            "item_emb": iemb,
            "co_user_emb": cuemb,
            "co_item_emb": ciemb,
            "co_user_bias": cub,
            "co_item_bias": cib,
        })
    return in_maps


def combine_outputs(cfg: Cfg, results: list[dict]) -> np.float32:
    c = cfg
    tot = np.zeros(N_SLOTS, np.float64)
    for r in results:
        tot += r["partials"].reshape(-1)[:N_SLOTS].astype(np.float64)
    ALPHA = 0.01
    BETA = 0.01
    loss = (tot[SLOT_RELU] / c.B
            + tot[SLOT_GLOVE_U] / (c.B * c.B)
            + tot[SLOT_GLOVE_I] / (c.B * c.B)
            + ALPHA * tot[SLOT_MSE_U] / (c.B * c.D)
            + BETA * tot[SLOT_MSE_I] / (c.B * c.D))
    return np.float32(loss)


def build_kernel(nc, cfg: Cfg):
    """Emit the whole SPMD program. Call inside no TileContext; this opens one."""
    c = cfg
    B = c.B

    # ---------------- DRAM I/O ----------------
    iw = nc.dram_tensor("iw", [c.NU, c.WIN], U8, kind="ExternalInput")
    itwin = nc.dram_tensor("itwin", [c.NI, c.WIN], U8, kind="ExternalInput")
    diag_off = nc.dram_tensor("diag_off", [128, B], F32, kind="ExternalInput")
    ids_c = nc.dram_tensor("ids_c", [128, 1], I32, kind="ExternalInput")
    pids_c = nc.dram_tensor("pids_c", [128, 1], I32, kind="ExternalInput")
    nids_c = nc.dram_tensor("nids_c", [128, c.NEG], I32, kind="ExternalInput")
    uball = nc.dram_tensor("uball", [128, c.MB], I32, kind="ExternalInput")
    iball = nc.dram_tensor("iball", [128, c.MB], I32, kind="ExternalInput")
    user_emb = nc.dram_tensor("user_emb", [c.NU, c.D], F32, kind="ExternalInput")
    item_emb = nc.dram_tensor("item_emb", [c.NI, c.D], F32, kind="ExternalInput")
    co_user_emb = nc.dram_tensor("co_user_emb", [c.NU, c.D], F32, kind="ExternalInput")
    co_item_emb = nc.dram_tensor("co_item_emb", [c.NI, c.D], F32, kind="ExternalInput")
    co_user_bias = nc.dram_tensor("co_user_bias", [c.NU, 1], F32, kind="ExternalInput")
    co_item_bias = nc.dram_tensor("co_item_bias", [c.NI, 1], F32, kind="ExternalInput")
    partials_out = nc.dram_tensor("partials", [1, N_SLOTS], F32, kind="ExternalOutput")

    with tile.TileContext(nc) as tc:
        with ExitStack() as ctx:
            sb = ctx.enter_context(tc.tile_pool(name="sb", bufs=1))
            sb_chunk = ctx.enter_context(tc.tile_pool(name="sb_chunk", bufs=2))
            sb_conv = ctx.enter_context(tc.tile_pool(name="sb_conv", bufs=3))
            sb_g = ctx.enter_context(tc.tile_pool(name="sb_g", bufs=3))
            sb_small = ctx.enter_context(tc.tile_pool(name="sb_small", bufs=1))
            dram = ctx.enter_context(tc.tile_pool(name="dram", bufs=1, space="DRAM"))

            for _rep in range(c.reps):
                _build_body(nc, tc, c, locals())
    return nc


def _gather_rows(nc, pool, table_ap, idx_tile, n_cols, dtype=F32, name=None):
    """Gather 128 rows of `table` (DRAM) by per-partition indices (SBUF
    [128,1] int32) into a [128, n_cols] SBUF tile."""
    out = pool.tile([128, n_cols], dtype, tag=name, name=name)
    nc.gpsimd.indirect_dma_start(
        out=out[:, :],
        out_offset=None,
        in_=table_ap,
        in_offset=bass.IndirectOffsetOnAxis(ap=idx_tile[:, :1], axis=0),
    )
    return out


def _build_body(nc, tc, c, env):
    B = c.B
    sb, sb_chunk, sb_conv, sb_g, sb_small = (
        env["sb"], env["sb_chunk"], env["sb_conv"], env["sb_g"], env["sb_small"])
    dram = env["dram"]
    iw, itwin = env["iw"], env["itwin"]

    NBLK = B // 512  # rhs N blocks of 512

    # ================= index tiles =================
    ids_t = sb.tile([128, 1], I32)
    nc.sync.dma_start(ids_t[:], env["ids_c"][:, :])
    pids_t = sb.tile([128, 1], I32)
    nc.sync.dma_start(pids_t[:], env["pids_c"][:, :])
    nids_t = sb.tile([128, c.NEG], I32)
    nc.sync.dma_start(nids_t[:], env["nids_c"][:, :])
    uball_t = sb.tile([128, c.MB], I32)
    nc.sync.dma_start(uball_t[:], env["uball"][:, :])
    iball_t = sb.tile([128, c.MB], I32)
    nc.sync.dma_start(iball_t[:], env["iball"][:, :])

    P_USER = bool(c.parts & 1)
    P_ITEM = bool(c.parts & 2)
    P_SMALL = bool(c.parts & 4)
    P_GLOVE = bool(c.parts & 8)
    # ====== both sides: row gather + convert + PE transpose -> G tiles ======
    # user side: gather user rows of I window shard  -> transpose -> (t, B)
    # item side: gather item rows of I^T window shard -> transpose -> (u, B)
    from concourse.masks import make_identity
    ident_bf = sb.tile([128, 128], BF16)
    make_identity(nc, ident_bf[:])

    def build_side(win_dram, idx_all, prefix, enabled, ps_tr):
        g = [sb.tile([128, B], BF16, tag=f"{prefix}{k}", name=f"{prefix}{k}")
             for k in range(c.KU if enabled else 0)]
        for b in range(c.MB if enabled else 0):
            row8 = sb_chunk.tile([128, c.WIN], U8, tag="row8", name="row8")
            nc.gpsimd.indirect_dma_start(
                out=row8[:, :], out_offset=None, in_=win_dram[:, :],
                in_offset=bass.IndirectOffsetOnAxis(ap=idx_all[:, b:b + 1],
                                                    axis=0))
            rbf = sb_chunk.tile([128, c.WIN], BF16, tag="rbf", name="rbf")
            if b % 2 == 0:
                nc.vector.tensor_copy(rbf[:], row8[:])
            else:
                nc.scalar.copy(rbf[:], row8[:])
            for k in range(c.KU):
                pt = ps_tr.tile([128, 128], BF16, tag="ps_tr", name="ps_tr")
                nc.tensor.transpose(out=pt[:],
                                    in_=rbf[:, 128 * k:128 * (k + 1)],
                                    identity=ident_bf[:])
                if (b + k) % 2 == 0:
                    nc.scalar.copy(g[k][:, 128 * b:128 * (b + 1)], pt[:])
                else:
                    nc.vector.tensor_copy(g[k][:, 128 * b:128 * (b + 1)], pt[:])
        return g

    with tc.tile_pool(name="ps_tr", bufs=4, space="PSUM") as ps_tr:
        gu = build_side(iw, uball_t, "gu", P_USER, ps_tr)
        gi = build_side(itwin, iball_t, "gi", P_ITEM, ps_tr)

    def conv_user(k):
        return gu[k]

    def conv_item(k):
        return gi[k]

    # ================= big GEMMs -> DRAM partials -> ReduceScatter ==========
    def side_gemm(nk, conv, rs_cat, base):
        ps_mm = env["ps_mm"]
        # two passes over m to fit 8 PSUM banks (4 m-blocks x NBLK n-blocks)
        m_per_pass = 8 // NBLK if NBLK <= 8 else 1
        assert c.MB % m_per_pass == 0
        for mp in range(c.MB // m_per_pass):
            ms = range(mp * m_per_pass, (mp + 1) * m_per_pass)
            acc = {(m, n): ps_mm.tile([128, 512], F32,
                                      tag=f"acc{(m % m_per_pass)}_{n}",
                                      name=f"acc{(m % m_per_pass)}_{n}")
                   for m in ms for n in range(NBLK)}
            for k in range(nk):
                g = conv(k)
                for m in ms:
                    for n in range(NBLK):
                        nc.tensor.matmul(
                            out=acc[(m, n)][:],
                            lhsT=g[:, 128 * m:128 * (m + 1)],
                            rhs=g[:, 512 * n:512 * (n + 1)],
                            start=(k == 0),
                            stop=(k == nk - 1),
                        )
            for m in ms:
                for n in range(NBLK):
                    stg = sb_conv.tile([128, 512], BF16, tag="mm_stage",
                                       name="mm_stage")
                    nc.vector.tensor_copy(stg[:], acc[(m, n)][:])
                    r0 = 256 * m + base
                    nc.sync.dma_start(
                        rs_cat[r0:r0 + 128, 512 * n:512 * (n + 1)],
                        stg[:])

    # Interleaved layout: rows [256m:256m+128) = user m-block, [+128:+256) =
    # item m-block, so an 8-way ReduceScatter hands rank c exactly its user
    # slice (rows 0:128 of output) and item slice (rows 128:256). bf16 is
    # exact: co-occurrence counts are integers <= ~200 < 256.
    rs_cat = dram.tile([2 * B, B], BF16)
    rs_out = dram.tile([2 * B // c.n_cores, B], BF16)
    with tc.tile_pool(name="ps_mm", bufs=1, space="PSUM") as ps_mm:
        env["ps_mm"] = ps_mm
        if P_USER:
            side_gemm(c.KU, conv_user, rs_cat, 0)
        if P_ITEM:
            side_gemm(c.KU, conv_item, rs_cat, 128)
    groups = [list(range(c.n_cores))]
    if c.no_cc:
        nc.sync.dma_start(rs_out[:, :], rs_cat[0:2 * B // c.n_cores, :])
    else:
        nc.gpsimd.collective_compute(
            "ReduceScatter", ALU.add, replica_groups=groups,
            ins=[rs_cat.opt()], outs=[rs_out.opt()])
    rs_out_u = rs_out[0:128, :]
    rs_out_i = rs_out[128:256, :]

    # ================= small gathers =================
    diag_t = sb.tile([128, B], F32)
    nc.sync.dma_start(diag_t[:], env["diag_off"][:, :])

    if not (P_SMALL or P_GLOVE):
        u_t = p_t = cu_t = ci_t = bu_t = bi_t = None
        cxu_all = cxi_all = bu_all = bi_all = None
    u_t = _gather_rows(nc, sb, env["user_emb"][:, :], ids_t, c.D, name="u_t") if (P_SMALL or P_GLOVE) else None
    if P_SMALL or P_GLOVE:
        p_t = _gather_rows(nc, sb, env["item_emb"][:, :], pids_t, c.D, name="p_t")
        cu_t = _gather_rows(nc, sb, env["co_user_emb"][:, :], ids_t, c.D, name="cu_t")
        ci_t = _gather_rows(nc, sb, env["co_item_emb"][:, :], pids_t, c.D, name="ci_t")
        bu_t = _gather_rows(nc, sb, env["co_user_bias"][:, :], ids_t, 1, name="bu_t")
        bi_t = _gather_rows(nc, sb, env["co_item_bias"][:, :], pids_t, 1, name="bi_t")

    # all-batch co_emb (for rhs of prod) and biases (for mb)
    def gather_all(table, idx_all, ncols, tag):
        tiles = []
        for b in range(c.MB):
            tiles.append(_gather_rows(nc, sb, table[:, :], idx_all[:, b:b + 1],
                                      ncols, name=f"{tag}{b}"))
        return tiles

    if P_GLOVE:
        cxu_all = gather_all(env["co_user_emb"], uball_t, c.D, "cxu")
        cxi_all = gather_all(env["co_item_emb"], iball_t, c.D, "cxi")
        bu_all = gather_all(env["co_user_bias"], uball_t, 1, "bua")
        bi_all = gather_all(env["co_item_bias"], iball_t, 1, "bia")

    ps_misc_cm = tc.tile_pool(name="ps_misc", bufs=1, space="PSUM")
    ps_misc = ps_misc_cm.__enter__()
    env["ps_misc_cm"] = ps_misc_cm  # keep alive; released at TileContext exit
    # identity for PE transpose
    from concourse.masks import make_identity
    ident = sb.tile([128, 128], F32)
    make_identity(nc, ident[:])

    # partials accumulate as columns of a [128, N_SLOTS] tile; ones-matmul at end
    parts = sb.tile([128, N_SLOTS], F32)
    nc.vector.memset(parts[:], 0.0)

    # ================= loss0 =================
    if P_SMALL:
        tmp = sb_small.tile([128, c.D], F32, tag="l0tmp")
        junk = sb_small.tile([128, c.D], F32, tag="l0junk")
        dpos = sb_small.tile([128, 1], F32, tag="dpos")
        nc.vector.tensor_sub(tmp[:], u_t[:], p_t[:])
        nc.scalar.activation(junk[:], tmp[:], AF.Square, accum_out=dpos[:])
        dneg = sb_small.tile([128, c.NEG], F32, tag="dneg")
        for l in range(c.NEG):
            n_t = _gather_rows(nc, sb_small, env["item_emb"][:, :],
                               nids_t[:, l:l + 1], c.D, name="n_t")
            t2 = sb_small.tile([128, c.D], F32, tag="l0tmp2")
            j2 = sb_small.tile([128, c.D], F32, tag="l0junk2")
            nc.vector.tensor_sub(t2[:], u_t[:], n_t[:])
            nc.scalar.activation(j2[:], t2[:], AF.Square, accum_out=dneg[:, l:l + 1])
        dmin = sb_small.tile([128, 1], F32, tag="dmin")
        nc.vector.tensor_reduce(dmin[:], dneg[:], axis=mybir.AxisListType.X, op=ALU.min)
        relu_r = sb_small.tile([128, 1], F32, tag="relu_r")
        nc.vector.tensor_sub(relu_r[:], dpos[:], dmin[:])
        nc.vector.tensor_scalar(parts[:, SLOT_RELU:SLOT_RELU + 1], relu_r[:],
                                float(MARGIN), 0.0, ALU.add, ALU.max)

        # ================= mse terms =================
        for (a_t, b_t, slot) in ((u_t, cu_t, SLOT_MSE_U), (p_t, ci_t, SLOT_MSE_I)):
            t2 = sb_small.tile([128, c.D], F32, tag="msetmp")
            j2 = sb_small.tile([128, c.D], F32, tag="msejunk")
            nc.vector.tensor_sub(t2[:], a_t[:], b_t[:])
            nc.scalar.activation(j2[:], t2[:], AF.Square,
                                 accum_out=parts[:, slot:slot + 1])

    # ================= glove sides =================
    ones_t = sb.tile([128, 1], F32)
    nc.vector.memset(ones_t[:], 1.0)
    ones_row = sb.tile([1, 128], F32)
    nc.vector.memset(ones_row[:], 1.0)

    def bcast128(val_ap, tag):
        '''broadcast a (1,1) SBUF value to a (128,1) SBUF tile via K=1 matmul'''
        bp = ps_misc.tile([128, 1], F32, tag="bc_ps", name="bc_ps")
        nc.tensor.matmul(out=bp[:], lhsT=ones_row[:], rhs=val_ap,
                         start=True, stop=True)
        bs = sb_small.tile([128, 1], F32, tag=tag, name=tag)
        nc.vector.tensor_copy(bs[:], bp[:])
        return bs

    def transpose128(src_ap, tag):
        """PE-transpose a [128,128] f32 block; returns SBUF tile."""
        pt = ps_misc.tile([128, 128], F32, tag="ps_t", name="ps_t")
        nc.tensor.transpose(out=pt[:], in_=src_ap, identity=ident[:])
        st = sb_small.tile([128, 128], F32, tag=tag, name=tag)
        nc.scalar.copy(st[:], pt[:])
        return st

    import os as _os
    KG = int(_os.environ.get("KGLOVE", "3"))

    def glove_side(co_rows_dram, x_c, b_c, x_all, b_all, slot):
        """co_rows_dram: [128,B] f32 summed co rows for this core's slice.
        x_c: [128,D] co emb rows for this core's slice (lhsT source).
        b_c: [128,1] bias rows. x_all/b_all: per-128-block gathered tiles."""
        nd = c.D // 128
        # build rhs co_xT blocks: nd tiles of [128, B]
        xT = [sb_small.tile([128, B], F32, tag=f"xT{d}", name=f"xT{d}") for d in range(nd)]
        for b in range(c.MB):
            for d in range(nd):
                t = transpose128(x_all[b][:, 128 * d:128 * (d + 1)], tag="xTblk")
                nc.vector.tensor_copy(xT[d][:, 128 * b:128 * (b + 1)], t[:])
        lhsT = [transpose128(x_c[:, 128 * d:128 * (d + 1)], tag=f"lhsT{d}")
                for d in range(nd)]  # noqa
        # prod rows: [128, B] fp32
        prod_ps = [ps_misc.tile([128, 512], F32, tag=f"prod{n}", name=f"prod{n}")
                   for n in range(NBLK)]
        for n in range(NBLK):
            for d in range(nd):
                nc.tensor.matmul(out=prod_ps[n][:], lhsT=lhsT[d][:],
                                 rhs=xT[d][:, 512 * n:512 * (n + 1)],
                                 start=(d == 0), stop=(d == nd - 1))
        if KG == 0:
            # checksum prod into parts: sum rows of prod
            pchk = sb_small.tile([128, 1], F32, tag="pchk", name="pchk")
            nc.vector.tensor_reduce(pchk[:], prod_ps[0][:],
                                    axis=mybir.AxisListType.X, op=ALU.add)
            nc.vector.tensor_copy(parts[:, slot:slot + 1], pchk[:])
            return
        # mb, mb2 from all-batch biases
        ball = sb_small.tile([128, c.MB], F32, tag="ball")
        for b in range(c.MB):
            nc.vector.tensor_copy(ball[:, b:b + 1], b_all[b][:])
        brow = sb_small.tile([128, 1], F32, tag="brow")
        nc.vector.tensor_reduce(brow[:], ball[:], axis=mybir.AxisListType.X,
                                op=ALU.add)
        b2row = sb_small.tile([128, 1], F32, tag="b2row")
        jb = sb_small.tile([128, c.MB], F32, tag="jb")
        nc.vector.tensor_tensor(jb[:], ball[:], ball[:], op=ALU.mult)
        nc.vector.tensor_reduce(b2row[:], jb[:], axis=mybir.AxisListType.X,
                                op=ALU.add)
        sums_ps = ps_misc.tile([1, 2], F32, tag="sums_ps")
        stk = sb_small.tile([128, 2], F32, tag="stk")
        nc.vector.tensor_copy(stk[:, 0:1], brow[:])
        nc.vector.tensor_copy(stk[:, 1:2], b2row[:])
        nc.tensor.matmul(out=sums_ps[:], lhsT=ones_t[:], rhs=stk[:],
                         start=True, stop=True)
        sums_sb = sb_small.tile([1, 2], F32, tag="sums_sb")
        nc.vector.tensor_copy(sums_sb[:], sums_ps[:])
        mb_b = bcast128(sums_sb[0:1, 0:1], "mb_b")
        mb2_b = bcast128(sums_sb[0:1, 1:2], "mb2_b")
        # mb = mb_b/B ; Kc = mb2/B - mb^2 ; bmb = b_c + mb
        mb_s = sb_small.tile([128, 1], F32, tag="mb_s")
        nc.vector.tensor_scalar_mul(mb_s[:], mb_b[:], 1.0 / B)
        kc = sb_small.tile([128, 1], F32, tag="kc")
        jk = sb_small.tile([128, 1], F32, tag="jk")
        nc.vector.tensor_tensor(jk[:], mb_s[:], mb_s[:], op=ALU.mult)
        nc.vector.tensor_scalar(kc[:], mb2_b[:], 1.0 / B, None, ALU.mult)
        nc.vector.tensor_sub(kc[:], kc[:], jk[:])
        bmb = sb_small.tile([128, 1], F32, tag="bmb")
        nc.vector.tensor_add(bmb[:], b_c[:], mb_s[:])
        if KG == 1:
            nc.vector.tensor_copy(parts[:, slot:slot + 1], bmb[:])
            return

        # C tile (bf16 from RS; convert)
        Cb = sb_small.tile([128, B], BF16, tag="gv_Cb")
        nc.sync.dma_start(Cb[:], co_rows_dram)
        C = sb_small.tile([128, B], F32, tag="gv_C")
        nc.vector.tensor_scalar_max(C[:], Cb[:], 1.0)
        L = sb_small.tile([128, B], F32, tag="gv_L")
        nc.scalar.activation(L[:], C[:], AF.Ln)
        w0 = sb_small.tile([128, B], F32, tag="gv_w0")
        exp_bias = sb_small.tile([128, 1], F32, tag="exp_bias", name="exp_bias")
        nc.vector.memset(exp_bias[:], float(-LAM * np.log(C_MAX)))
        nc.scalar.activation(w0[:], L[:], AF.Exp, scale=float(LAM),
                             bias=exp_bias[:])
        w1 = sb_small.tile([128, B], F32, tag="gv_w1")
        nc.vector.scalar_tensor_tensor(w1[:], w0[:], 1.0, diag_t[:],
                                       op0=ALU.min, op1=ALU.mult)
        m1 = sb_small.tile([128, B], F32, tag="gv_m1")
        nc.vector.tensor_scalar(m1[:], C[:], -1.0, 1.0, ALU.add, ALU.min)
        S2 = sb_small.tile([128, 1], F32, tag="S2")
        w = sb_small.tile([128, B], F32, tag="gv_w")
        nc.vector.tensor_tensor(w[:], w1[:], m1[:], op=ALU.mult)
        nc.vector.tensor_reduce(S2[:], w[:], axis=mybir.AxisListType.X,
                                op=ALU.add)
        if KG == 2:
            nc.vector.tensor_copy(parts[:, slot:slot + 1], S2[:])
            return
        # A2 = (prod + bmb) - L   (prod read from PSUM)
        A2 = sb_small.tile([128, B], F32, tag="gv_A2")
        for n in range(NBLK):
            nc.vector.scalar_tensor_tensor(
                A2[:, 512 * n:512 * (n + 1)], prod_ps[n][:], bmb[:],
                L[:, 512 * n:512 * (n + 1)], op0=ALU.add, op1=ALU.subtract)
        X = sb_small.tile([128, B], F32, tag="gv_X")
        nc.vector.tensor_tensor(X[:], A2[:], A2[:], op=ALU.mult)
        S1 = sb_small.tile([128, 1], F32, tag="S1")
        XJ = sb_small.tile([128, B], F32, tag="gv_XJ")
        nc.vector.tensor_tensor(XJ[:], X[:], w[:], op=ALU.mult)
        nc.vector.tensor_reduce(S1[:], XJ[:], axis=mybir.AxisListType.X,
                                op=ALU.add)
        nc.vector.scalar_tensor_tensor(parts[:, slot:slot + 1], S2[:], kc[:],
                                       S1[:], op0=ALU.mult, op1=ALU.add)

    if P_GLOVE and P_USER:
        glove_side(rs_out_u, cu_t, bu_t, cxu_all, bu_all, SLOT_GLOVE_U)
    if P_GLOVE and P_ITEM:
        glove_side(rs_out_i, ci_t, bi_t, cxi_all, bi_all, SLOT_GLOVE_I)

    # ================= final partition reduction =================
    fin_ps = ps_misc.tile([1, N_SLOTS], F32, tag="fin_ps")
    nc.tensor.matmul(out=fin_ps[:], lhsT=ones_t[:], rhs=parts[:],
                     start=True, stop=True)
    fin_sb = sb_small.tile([1, N_SLOTS], F32, tag="fin_sb")
    nc.vector.tensor_copy(fin_sb[:], fin_ps[:])
    nc.sync.dma_start(env["partials_out"][:, :], fin_sb[:])


# ======================================================================
# Self-contained runner: kernel(**inputs) -> full-shape output (scalar)
# ======================================================================
import sys as _sys
if "/opt/trn_rl_repo" not in _sys.path:
    _sys.path.insert(0, "/opt/trn_rl_repo")

_COMPILED = {}


def _get_compiled(cfg_key=None):
    import concourse.bacc as bacc
    import os as _os
    if "nc" not in _COMPILED:
        cfg = Cfg(parts=int(_os.environ.get("KPARTS", "15")),
                  reps=int(_os.environ.get("KREPS", "1")))
        nc = bacc.Bacc("TRN2", target_bir_lowering=False, debug=False,
                       num_devices=cfg.n_cores)
        build_kernel(nc, cfg)
        nc.compile()
        _COMPILED["nc"] = nc
        _COMPILED["cfg"] = cfg
    return _COMPILED["nc"], _COMPILED["cfg"]


def kernel(**inputs):
    from concourse.bass_utils import run_bass_kernel_spmd
    nc, cfg = _get_compiled()
    in_maps = build_host_inputs(cfg, inputs)
    res = run_bass_kernel_spmd(nc, in_maps, list(range(cfg.n_cores)))
    return combine_outputs(cfg, res.results)

